# revision 1
# baseline (speedup 1.0000x reference)
"""Trainium2 Bass kernel for nn_MetaLSTMDetector: 2-layer LSTM (H=256) over
sliding 4-tap windows of y[64, 4096], projected to [64, 4096, 2].

Strategy: pure data parallelism — batch 64 split as 8 sequences per NeuronCore;
LSTM weights replicated; the T=4096 scan runs locally on each core.

Per-core layout (B=8 local sequences):
- Gate order permuted to [i, f, o, g] on host so all sigmoid gates are
  contiguous (one ACT op) and tanh(g) is one more.
- Everything is feature-major: [128 partitions = feature%128,
  free = (j=feature//128, t, b)], so the elementwise LSTM math uses all
  128 lanes of the Vector/Scalar engines.
- Per chunk of Tc=8 steps a PSUM bank [128, 512] accumulates the gates:
  phase A (tensor engine, K=5 matmul over the 4 window taps + a ones-row
  carrying the bias) fills the input-side contribution for all 8 steps at
  once; the recurrent W_hh @ h_t matmuls then accumulate into the same
  columns step by step (weight-stationary: out = W_chunk.T-stationary,
  h streamed, so the gates land pre-transposed).
- Layer 1's input contribution W_ih1 @ h0 is batched per chunk (phase C),
  so the per-step burst of each cell is only 16 LDWEIGHTS+MATMUL pairs.
- Output projection W_out (phase E) is batched per chunk and DMA'd out.
"""
import os, sys

for _p in ("/opt/trn_rl_repo", "/root/.axon_site/_ro/trn_rl_repo"):
    if os.path.isdir(_p) and _p not in sys.path:
        sys.path.insert(0, _p)

import numpy as np
import concourse.bass as bass
import concourse.mybir as mybir
import concourse.tile as tile
import concourse.bacc as bacc
from concourse.bass_utils import run_bass_kernel_spmd

f32 = mybir.dt.float32
bf16 = mybir.dt.bfloat16
AF = mybir.ActivationFunctionType

H = 256
B = 8           # sequences per core
TC = 8          # steps per chunk
CPI = 2         # chunks per loop iteration
N_CORES = 8
PAD = -100.0
PERM = np.r_[0:256, 256:512, 768:1024, 512:768]   # [i, f, o, g]

LAST_EXEC_TIME_NS = None
_NC_CACHE = {}


def _build_nc(n_iter, use_bf16=False):
    T = n_iter * CPI * TC
    wdt = bf16 if use_bf16 else f32
    nc = bacc.Bacc()

    y5c_d = nc.dram_tensor("y5c", [5, T * B], f32, kind="ExternalInput")
    w05_d = nc.dram_tensor("w05", [5, 4 * H], f32, kind="ExternalInput")
    whh0_d = nc.dram_tensor("whh0", [H, 4 * H], wdt, kind="ExternalInput")
    wih1_d = nc.dram_tensor("wih1", [H, 4 * H], wdt, kind="ExternalInput")
    whh1_d = nc.dram_tensor("whh1", [H, 4 * H], wdt, kind="ExternalInput")
    b1e_d = nc.dram_tensor("b1e", [128, TC * 8 * B], f32, kind="ExternalInput")
    wout_d = nc.dram_tensor("wout", [H, 2], wdt, kind="ExternalInput")
    bout_d = nc.dram_tensor("bout", [2, 1], f32, kind="ExternalInput")
    out_d = nc.dram_tensor("out", [2, T * B], f32, kind="ExternalOutput")

    JB = TC * B        # 64 cols per j-block
    GW = 8 * JB        # 512: gin tile width (one PSUM bank)
    HW = 2 * JB        # 128: H tile width

    with tile.TileContext(nc) as tc:
        with (
            tc.tile_pool(name="const", bufs=1) as cp,
            tc.tile_pool(name="psum", bufs=1, space="PSUM") as pp,
        ):
            sY = cp.tile([5, T * B], f32, name="sY")
            sW05 = cp.tile([5, 4 * H], f32, name="sW05")
            sWhh0 = [cp.tile([128, 4 * H], wdt, name=f"sWhh0{k}") for k in range(2)]
            sWih1 = [cp.tile([128, 4 * H], wdt, name=f"sWih1{k}") for k in range(2)]
            sWhh1 = [cp.tile([128, 4 * H], wdt, name=f"sWhh1{k}") for k in range(2)]
            sB1e = cp.tile([128, GW], f32, name="sB1e")
            sWout = [cp.tile([128, 2], wdt, name=f"sWout{k}") for k in range(2)]
            sBout = cp.tile([2, 1], f32, name="sBout")

            H0 = [cp.tile([128, HW], wdt, name=f"H0{h}") for h in range(2)]
            H1 = [cp.tile([128, HW], wdt, name=f"H1{h}") for h in range(2)]
            c0 = cp.tile([128, 16], f32, name="c0")
            c1 = cp.tile([128, 16], f32, name="c1")
            sig0 = [cp.tile([128, 48], f32, name=f"sig0{p}") for p in range(2)]
            sig1 = [cp.tile([128, 48], f32, name=f"sig1{p}") for p in range(2)]
            g0s = [cp.tile([128, 16], f32, name=f"g0s{p}") for p in range(2)]
            g1s = [cp.tile([128, 16], f32, name=f"g1s{p}") for p in range(2)]
            t0s = [cp.tile([128, 16], f32, name=f"t0s{p}") for p in range(2)]
            t1s = [cp.tile([128, 16], f32, name=f"t1s{p}") for p in range(2)]
            m1s = [cp.tile([128, 16], f32, name=f"m1s{p}") for p in range(2)]
            m2s = [cp.tile([128, 16], f32, name=f"m2s{p}") for p in range(2)]
            n1s = [cp.tile([128, 16], f32, name=f"n1s{p}") for p in range(2)]
            n2s = [cp.tile([128, 16], f32, name=f"n2s{p}") for p in range(2)]
            outSb = [cp.tile([2, JB], f32, name=f"outSb{h}") for h in range(2)]

            gin0 = [pp.tile([128, GW], f32, name=f"gin0{h}") for h in range(2)]
            gin1 = [pp.tile([128, GW], f32, name=f"gin1{h}") for h in range(2)]
            pout = [pp.tile([2, JB], f32, name=f"pout{h}") for h in range(2)]

            nc.sync.dma_start(sY[:], y5c_d[:])
            nc.sync.dma_start(sW05[:], w05_d[:])
            for k in range(2):
                nc.sync.dma_start(sWhh0[k][:], whh0_d[128 * k:128 * (k + 1), :])
                nc.sync.dma_start(sWih1[k][:], wih1_d[128 * k:128 * (k + 1), :])
                nc.sync.dma_start(sWhh1[k][:], whh1_d[128 * k:128 * (k + 1), :])
                nc.sync.dma_start(sWout[k][:], wout_d[128 * k:128 * (k + 1), :])
            nc.sync.dma_start(sB1e[:], b1e_d[:])
            nc.sync.dma_start(sBout[:], bout_d[:])
            for h in range(2):
                nc.vector.memset(H0[h][:], 0.0)
                nc.vector.memset(H1[h][:], 0.0)
            nc.vector.memset(c0[:], 0.0)
            nc.vector.memset(c1[:], 0.0)

            def cell_step(ginT, Hc, Hp, cT, Wk, sigT, gT, tT, m1T, m2T, t):
                Hsrc, po = (Hp, (TC - 1) * 8) if t == 0 else (Hc, (t - 1) * 8)
                for j in range(8):
                    for k in range(2):
                        nc.tensor.matmul(
                            ginT[:, j * JB + t * 8: j * JB + t * 8 + 8],
                            Wk[k][:, j * 128:(j + 1) * 128],
                            Hsrc[:, k * JB + po: k * JB + po + 8],
                            start=False, stop=(j == 7 and k == 1),
                            skip_group_check=True,
                        )
                ginR = ginT.rearrange("p (j x) -> p j x", j=8)
                nc.scalar.activation(sigT[:].rearrange("p (j x) -> p j x", j=6),
                                     ginR[:, 0:6, t * 8:t * 8 + 8], AF.Sigmoid)
                nc.scalar.activation(gT[:].rearrange("p (j x) -> p j x", j=2),
                                     ginR[:, 6:8, t * 8:t * 8 + 8], AF.Tanh)
                nc.vector.tensor_mul(m1T[:], sigT[:, 16:32], cT[:])   # f*c
                nc.vector.tensor_mul(m2T[:], sigT[:, 0:16], gT[:])    # i*g~
                nc.vector.tensor_add(cT[:], m1T[:], m2T[:])
                nc.scalar.activation(tT[:], cT[:], AF.Tanh)
                HcR = Hc.rearrange("p (j x) -> p j x", j=2)
                nc.vector.tensor_mul(HcR[:, :, t * 8:t * 8 + 8],
                                     sigT[:].rearrange("p (j x) -> p j x", j=6)[:, 4:6, :],
                                     tT[:].rearrange("p (j x) -> p j x", j=2))

            def half_body(coff, h):
                ginA, ginB = gin0[h], gin1[h]
                H0c, H0p = H0[h], H0[1 - h]
                H1c, H1p = H1[h], H1[1 - h]
                for j in range(8):
                    nc.tensor.matmul(
                        ginA[:, j * JB:(j + 1) * JB],
                        sW05[:, j * 128:(j + 1) * 128],
                        sY[:, bass.ds(coff, JB)],
                        start=(j == 0), stop=False, skip_group_check=True,
                    )
                for t in range(TC):
                    cell_step(ginA, H0c, H0p, c0, sWhh0,
                              sig0[t % 2], g0s[t % 2], t0s[t % 2],
                              m1s[t % 2], m2s[t % 2], t)
                for j in range(8):
                    for k in range(2):
                        nc.tensor.matmul(
                            ginB[:, j * JB:(j + 1) * JB],
                            sWih1[k][:, j * 128:(j + 1) * 128],
                            H0c[:, k * JB:(k + 1) * JB],
                            start=(j == 0 and k == 0), stop=(k == 1),
                            skip_group_check=True,
                        )
                nc.vector.tensor_add(ginB[:], ginB[:], sB1e[:])
                for t in range(TC):
                    cell_step(ginB, H1c, H1p, c1, sWhh1,
                              sig1[t % 2], g1s[t % 2], t1s[t % 2],
                              n1s[t % 2], n2s[t % 2], t)
                nc.tensor.matmul(pout[h][:], sWout[0][:], H1c[:, 0:JB],
                                 start=True, stop=False, skip_group_check=True)
                nc.tensor.matmul(pout[h][:], sWout[1][:], H1c[:, JB:2 * JB],
                                 start=False, stop=True, skip_group_check=True)
                nc.vector.tensor_scalar_add(outSb[h][:], pout[h][:], sBout[:, 0:1])
                nc.sync.dma_start(out_d[:, bass.ds(coff, JB)], outSb[h][:])

            if n_iter == 1:
                for h in range(CPI):
                    half_body(h * JB, h)
            else:
                with tc.For_i(0, n_iter, 1,
                              hint_engines=(mybir.EngineType.PE,)) as it:
                    base = it * (CPI * JB)
                    for h in range(CPI):
                        half_body(base + h * JB, h)

    nc.compile()
    return nc


def _prep_core_inputs(y_local, W_ih0, W_hh0, b_ih0, b_hh0,
                      W_ih1, W_hh1, b_ih1, b_hh1, W_out, b_out,
                      use_bf16=False):
    import ml_dtypes
    wdt = ml_dtypes.bfloat16 if use_bf16 else np.float32
    Bl, T = y_local.shape

    yp = np.concatenate(
        [np.full((Bl, 3), PAD, np.float32), y_local.astype(np.float32)], axis=1)
    y5c = np.empty((5, T * Bl), np.float32)
    for k in range(4):
        y5c[k] = yp[:, k:k + T].T.reshape(-1)
    y5c[4] = 1.0

    w05 = np.empty((5, 1024), np.float32)
    w05[0:4] = W_ih0.T[:, PERM]
    w05[4] = (b_ih0 + b_hh0)[PERM]

    whh0 = np.ascontiguousarray(W_hh0[PERM].T).astype(wdt)
    wih1 = np.ascontiguousarray(W_ih1[PERM].T).astype(wdt)
    whh1 = np.ascontiguousarray(W_hh1[PERM].T).astype(wdt)

    b1 = (b_ih1 + b_hh1)[PERM]
    b1e = np.empty((128, 8, TC * 8), np.float32)
    for j in range(8):
        b1e[:, j, :] = b1[j * 128:(j + 1) * 128][:, None]
    b1e = b1e.reshape(128, 8 * TC * 8)

    wout = np.ascontiguousarray(W_out.T).astype(wdt)
    bout = b_out.reshape(2, 1).astype(np.float32)

    return {"y5c": y5c, "w05": w05, "whh0": whh0, "wih1": wih1,
            "whh1": whh1, "b1e": b1e, "wout": wout, "bout": bout}


def kernel(y, W_ih0, W_hh0, b_ih0, b_hh0, W_ih1, W_hh1, b_ih1, b_hh1,
           W_out, b_out):
    global LAST_EXEC_TIME_NS
    y = np.asarray(y, np.float32)
    args = [np.asarray(a, np.float32) for a in
            (W_ih0, W_hh0, b_ih0, b_hh0, W_ih1, W_hh1, b_ih1, b_hh1,
             W_out, b_out)]
    Bfull, T = y.shape
    assert Bfull == N_CORES * B and T % (CPI * TC) == 0
    n_iter = T // (CPI * TC)
    use_bf16 = os.environ.get("BASS_LSTM_BF16", "0") == "1"

    key = (n_iter, use_bf16)
    if key not in _NC_CACHE:
        _NC_CACHE[key] = _build_nc(n_iter, use_bf16=use_bf16)
    nc = _NC_CACHE[key]

    in_maps = [_prep_core_inputs(y[B * c:B * (c + 1)], *args,
                                 use_bf16=use_bf16) for c in range(N_CORES)]
    trace = os.environ.get("BASS_LSTM_TRACE", "0") == "1"
    res = run_bass_kernel_spmd(nc, in_maps, core_ids=list(range(N_CORES)),
                               trace=trace)
    if trace:
        LAST_EXEC_TIME_NS = res.exec_time_ns

    out = np.empty((Bfull, T, 2), np.float32)
    for c in range(N_CORES):
        o = res.results[c]["out"].reshape(2, T, B).transpose(2, 1, 0)
        out[B * c:B * (c + 1)] = o
    return out
